# revision 5
# baseline (speedup 1.0000x reference)
"""ContrastiveHardestNegativeLoss on 8 Trainium2 NeuronCores (Bass/Tile).

Strategy (per sharding hint): shard the positive-pair (row) dimension of the
P x M distance matrices across the 8 cores. Each core receives:
  - its slice of the gathered pos features, transposed + augmented:
      lhs[d, i] = posF[i, d] for d < 32,  lhs[32, i] = 1.0
  - the full gathered sub features, transposed, scaled and augmented:
      rhs[d, c] = -2 * subF[c, d] for d < 32,  rhs[32, c] = |subF[c]|^2
  so a single PE matmul produces q[i, c] = |subF[c]|^2 - 2 <posF[i], subF[c]>,
  i.e. the squared distance minus the (row-constant) |posF[i]|^2 term.

The hardest-negative terms are exactly zero whenever every distance exceeds
NEG_THRESH (true with huge margin here: min distance ~2.9 vs 1.4), in which
case the pair-mask cannot affect the result. The kernel therefore only needs
a *certificate* that no squared distance falls below a conservative threshold
TH = 3.0 > NEG_THRESH^2 + bf16 error margin; if the certificate fails it
falls back to an exact host recompute.

Both PSUM-egress paths are just zero-certificates, so every 1024-col PSUM
granule can be consumed independently by EITHER of the two engines with a
PSUM read port, with no cross-engine dependency:
  - VectorE: stock tensor_reduce(min) PSUM -> [128,1] min column; the
    epilogue adds back |p_i|^2 and relu-tests against TH.
  - ScalarE: one activation relu((TH - |p_i|^2) - q) with per-partition bias
    and SUM-accumulate: a zero sum proves every d2 in the granule > TH.
Granules are interleaved V:S = 119:137 (Bresenham), matching the engines'
rates (DVE 0.96 GHz, Act 1.2 GHz), so both PSUM ports stay saturated; this
is the hardware egress roofline (PE matmul at 109us/core is below it).
Output per core: [pos_sum, flag]; flag must be exactly 0.
"""

import numpy as np

import concourse.bacc as bacc
import concourse.mybir as mybir
import concourse.tile as tile
from concourse.bass_utils import run_bass_kernel_spmd

N_CORES = 8
N_PTS = 100000
D = 32
P = 16384
M = 8192
P_LOC = P // N_CORES            # 2048 rows per core
RT = P_LOC // 128               # 16 row tiles
CHUNK = 1024                    # q columns per PSUM granule (2 banks)
NCH = M // CHUNK                # 8 chunks
KA = D + 1                      # contraction dim with augmentation row
POS_THRESH = 0.1
NEG_THRESH = 1.4
TH = 3.0                        # certificate threshold on d^2 (vs
                                # NEG_THRESH^2=1.96; margin covers bf16 error)

NGRAN = RT * 2 * NCH            # 256 granules per core
NV = 119                        # granules consumed by VectorE min-reduce
NS = NGRAN - NV                 # granules consumed by ScalarE relu-cert

F32 = mybir.dt.float32
BF16 = mybir.dt.bfloat16
AX = mybir.AxisListType
ALU = mybir.AluOpType
ACT = mybir.ActivationFunctionType

_CACHED_NC = None
LAST_RESULTS = None            # test.py reads .exec_time_ns after a traced run


def _register_const(nc, value):
    t = nc.alloc_sbuf_tensor(f"const-float32-{value}", [128, 1], F32)
    nc.gpsimd.memset(t.ap(), value)
    nc.const_aps.aps[(F32, value)] = t.ap()


def _is_v(g):
    """Bresenham V/S interleave at ratio NV/NGRAN."""
    return ((g + 1) * NV) // NGRAN > (g * NV) // NGRAN


def _build_nc():
    nc = bacc.Bacc("TRN2", debug=False, target_bir_lowering=False,
                   num_devices=N_CORES)
    # Const APs are memset once here and first read much later in the
    # epilogue; no barrier needed.
    for v in (-POS_THRESH, TH):
        _register_const(nc, v)
    # fp32 pos-pair operands (positive loss needs full precision);
    # bf16 copies feed the distance-matrix matmuls (fp32 PE matmul streams at
    # 1/4 rate; bf16 error on a distance is ~0.05 vs a 1.5 threshold margin).
    lhsA = nc.dram_tensor("lhsA", [KA, P_LOC], F32, kind="ExternalInput").ap()
    lhsB = nc.dram_tensor("lhsB", [KA, P_LOC], F32, kind="ExternalInput").ap()
    lhsAh = nc.dram_tensor("lhsAh", [KA, P_LOC], BF16, kind="ExternalInput").ap()
    lhsBh = nc.dram_tensor("lhsBh", [KA, P_LOC], BF16, kind="ExternalInput").ap()
    rhsAh = nc.dram_tensor("rhsAh", [KA, M], BF16, kind="ExternalInput").ap()
    rhsBh = nc.dram_tensor("rhsBh", [KA, M], BF16, kind="ExternalInput").ap()
    pnA = nc.dram_tensor("pnA", [128, RT], F32, kind="ExternalInput").ap()
    pnB = nc.dram_tensor("pnB", [128, RT], F32, kind="ExternalInput").ap()
    bthA = nc.dram_tensor("bthA", [128, RT], F32, kind="ExternalInput").ap()
    bthB = nc.dram_tensor("bthB", [128, RT], F32, kind="ExternalInput").ap()
    ones = nc.dram_tensor("ones", [128, 1], F32, kind="ExternalInput").ap()
    outd = nc.dram_tensor("out", [1, 2], F32, kind="ExternalOutput").ap()

    with tile.TileContext(nc) as tc:
        with (
            tc.tile_pool(name="ops", bufs=1) as ops,
            tc.tile_pool(name="wk", bufs=2) as wk,
            tc.tile_pool(name="ps", bufs=4, space="PSUM") as ps,
        ):
            t_lhsA = ops.tile([KA, P_LOC], F32, tag="lhsA")
            t_lhsB = ops.tile([KA, P_LOC], F32, tag="lhsB")
            # bf16 operands are loaded TWICE: rows 0..32 and rows 64..96, so
            # two row-tiles' matmuls can run concurrently on the two 64-row
            # groups of the PE array (K=33 rounds up to a 64-row group).
            t_lhsAh = ops.tile([128, P_LOC], BF16, tag="lhsAh")
            t_lhsBh = ops.tile([128, P_LOC], BF16, tag="lhsBh")
            t_rhsAh = ops.tile([128, M], BF16, tag="rhsAh")
            t_rhsBh = ops.tile([128, M], BF16, tag="rhsBh")
            t_pnAB = ops.tile([128, 2 * RT], F32, tag="pnAB")
            t_bthA = ops.tile([128, RT], F32, tag="bthA")
            t_bthB = ops.tile([128, RT], F32, tag="bthB")
            t_ones = ops.tile([128, 1], F32, tag="ones")
            t_racc = ops.tile([128, NS + 1], F32, tag="racc")
            t_vmin = ops.tile([128, NGRAN], F32, tag="vmin")

            # operand loads strictly in consumption order, chunk 0's rhs and
            # the first weights first so the first matmul can start early;
            # the bulk of each tensor goes in one coarse descriptor.
            sl0 = slice(0, CHUNK)
            slr = slice(CHUNK, M)
            for base in (0, 64):
                nc.sync.dma_start(t_rhsAh[base:base + KA, sl0], rhsAh[:, sl0])
            for base in (0, 64):
                nc.sync.dma_start(t_lhsAh[base:base + KA, :], lhsAh[:])
            for base in (0, 64):
                nc.sync.dma_start(t_rhsAh[base:base + KA, slr], rhsAh[:, slr])
            for base in (0, 64):
                nc.sync.dma_start(t_lhsBh[base:base + KA, :], lhsBh[:])
            for base in (0, 64):
                nc.sync.dma_start(t_rhsBh[base:base + KA, :], rhsBh[:])
            # aux + fp32 operands go over the (otherwise idle) GpSimd DMA
            # ring so they don't queue behind the bf16 stream on the sync
            # ring; the fp32 lhs feed the pos-path prep that runs early.
            nc.gpsimd.dma_start(t_lhsA[:], lhsA[:])
            nc.gpsimd.dma_start(t_lhsB[:], lhsB[:])
            nc.gpsimd.dma_start(t_bthA[:], bthA[:])
            nc.gpsimd.dma_start(t_bthB[:], bthB[:])
            nc.gpsimd.dma_start(t_pnAB[:, 0:RT], pnA[:])
            nc.gpsimd.dma_start(t_pnAB[:, RT:2 * RT], pnB[:])
            nc.gpsimd.dma_start(t_ones[:], ones[:])

            # pos-path prep (VectorE sub + ScalarE square) early: fills the
            # consumers' startup idle while the first bf16 chunks stream in.
            t_onesh = ops.tile([KA, 1], BF16, tag="onesh")
            nc.gpsimd.memset(t_onesh[:], 1.0)
            t_dif = ops.tile([KA, P_LOC], F32, tag="dif")
            nc.vector.tensor_tensor(t_dif[:], t_lhsA[:], t_lhsB[:], ALU.subtract)
            t_difsq = ops.tile([KA, P_LOC], BF16, tag="difsq")
            nc.scalar.activation(t_difsq[:], t_dif[:], ACT.Square)

            # racc columns beyond rcol must stay zero; vmin columns of
            # S-assigned granules must not poison the epilogue min.
            nc.gpsimd.memset(t_racc[:], 0.0)
            nc.gpsimd.memset(t_vmin[:], 3.0e38)

            t_outsb = wk.tile([1, 2], F32, tag="outsb")

            # ---- the two distance matrices ----
            # Row-tiles processed in pairs (PE row-groups 0 and 64). Each
            # 1024-col granule is consumed from PSUM exactly once, by the
            # engine the Bresenham schedule assigns.
            g = 0
            rcol = 0
            for pr in range(RT // 2):
                for mi, (t_lhs, t_rhs, t_bth) in enumerate((
                    (t_lhsAh, t_rhsAh, t_bthA),
                    (t_lhsBh, t_rhsBh, t_bthB),
                )):
                    for k in range(NCH):
                        # emit the two halves' matmuls interleaved by 512-col
                        # piece so consecutive PE ops alternate row-groups
                        # (lets each LDWEIGHTS pull ahead during the other
                        # group's matmul).
                        qs = {}
                        for half in (0, 1):
                            qs[half] = ps.tile([128, CHUNK], F32, tag="q",
                                               name=f"q{half}")
                        for j in range(CHUNK // 512):
                            for half in (0, 1):
                                r = 2 * pr + half
                                base = 64 * half
                                w = t_lhs[base:base + KA,
                                          r * 128:(r + 1) * 128]
                                c0 = k * CHUNK + j * 512
                                nc.tensor.matmul(
                                    qs[half][:, j * 512:(j + 1) * 512], w,
                                    t_rhs[base:base + KA, c0:c0 + 512])
                        for half in (0, 1):
                            r = 2 * pr + half
                            q = qs[half]
                            if _is_v(g):
                                col = mi * 128 + r * 8 + k
                                nc.vector.tensor_reduce(
                                    out=t_vmin[:, col:col + 1], in_=q[:],
                                    axis=AX.X, op=ALU.min)
                            else:
                                junk = wk.tile([128, CHUNK], BF16,
                                               tag="junk", bufs=2)
                                nc.scalar.activation(
                                    junk[:], q[:], ACT.Relu,
                                    bias=t_bth[:, r:r + 1], scale=-1.0,
                                    accum_out=t_racc[:, rcol:rcol + 1])
                                rcol += 1
                            g += 1
                    # positive-pair loss, emitted early in the stream so its
                    # ScalarE relu lands inside the main span (not the tail):
                    # relu(sum_d (p0-p1)^2 - 0.1) summed via accum_out. The
                    # per-pair squares are summed on the PE via a ones-matmul
                    # in bf16 (rounding perturbs the loss ~1e-5 relative).
                    if pr == 0 and mi == 1:
                        t_posacc = ops.tile([1, 2], F32, tag="posacc")
                        for h in range(2):
                            pp = ps.tile([128, CHUNK], F32, tag="q",
                                         name="pp")
                            for j in range(2):
                                c0 = (2 * h + j) * 512
                                nc.tensor.matmul(
                                    pp[0:1, j * 512:(j + 1) * 512],
                                    t_onesh[:], t_difsq[:, c0:c0 + 512])
                            junkp = wk.tile([1, CHUNK], BF16, tag="junkp")
                            nc.scalar.activation(
                                junkp[:], pp[0:1, :], ACT.Relu,
                                bias=-POS_THRESH,
                                accum_out=t_posacc[0:1, h:h + 1])

            # ---- fused epilogue over both matrices: per-row min -> d2 ->
            # relu(TH - d2), combined with the S-path relu certificate into
            # one nonnegative flag that must be exactly 0.
            nc.vector.tensor_reduce(
                out=t_outsb[0:1, 0:1], in_=t_posacc[:], axis=AX.X,
                op=ALU.add)
            minq = wk.tile([128, 2 * RT], F32, tag="minq")
            nc.vector.tensor_reduce(
                out=minq[:],
                in_=t_vmin.rearrange("p (m k) -> p m k", k=NCH),
                axis=AX.X, op=ALU.min)
            d2 = wk.tile([128, 2 * RT], F32, tag="d2")
            nc.vector.tensor_tensor(d2[:], minq[:], t_pnAB[:], ALU.add)
            junkq = wk.tile([128, 2 * RT], BF16, tag="junkq")
            t_flag = wk.tile([128, 1], F32, tag="flag")
            nc.scalar.activation(junkq[:], d2[:], ACT.Relu, bias=TH,
                                 scale=-1.0, accum_out=t_flag[:])
            rs = wk.tile([128, 1], F32, tag="rs")
            nc.vector.tensor_reduce(out=rs[:], in_=t_racc[:], axis=AX.X,
                                    op=ALU.add)
            comb = wk.tile([128, 1], F32, tag="comb")
            nc.vector.tensor_tensor(comb[:], t_flag[:], rs[:], ALU.add)
            fp = ps.tile([1, 1], F32, tag="q")
            nc.tensor.matmul(fp[:], comb[:], t_ones[:])
            nc.scalar.copy(t_outsb[0:1, 1:2], fp[0:1, 0:1])

            nc.sync.dma_start(outd[:], t_outsb[:])

    nc.compile()
    return nc


def _prep_inputs(F0, F1, matches, sel0, sel1):
    posF0 = F0[matches[:, 0]]
    posF1 = F1[matches[:, 1]]
    subF0 = F0[sel0]
    subF1 = F1[sel1]
    import ml_dtypes

    bf16 = ml_dtypes.bfloat16
    ones_col = np.ones((1, P_LOC), np.float32)
    rhsA = np.ascontiguousarray(
        np.concatenate([-2.0 * subF1.T, (subF1 * subF1).sum(1)[None, :]], 0),
        dtype=np.float32)
    rhsB = np.ascontiguousarray(
        np.concatenate([-2.0 * subF0.T, (subF0 * subF0).sum(1)[None, :]], 0),
        dtype=np.float32)
    rhsAh = np.ascontiguousarray(rhsA, dtype=bf16)
    rhsBh = np.ascontiguousarray(rhsB, dtype=bf16)
    ones_in = np.ones((128, 1), np.float32)
    in_maps = []
    for c in range(N_CORES):
        sl = slice(c * P_LOC, (c + 1) * P_LOC)
        p0, p1 = posF0[sl], posF1[sl]
        lhsA = np.ascontiguousarray(
            np.concatenate([p0.T, ones_col], 0), dtype=np.float32)
        lhsB = np.ascontiguousarray(
            np.concatenate([p1.T, ones_col], 0), dtype=np.float32)
        pnA_c = np.ascontiguousarray(
            (p0 * p0).sum(1).reshape(RT, 128).T, dtype=np.float32)
        pnB_c = np.ascontiguousarray(
            (p1 * p1).sum(1).reshape(RT, 128).T, dtype=np.float32)
        in_maps.append({
            "lhsA": lhsA,
            "lhsB": lhsB,
            "lhsAh": np.ascontiguousarray(lhsA, dtype=bf16),
            "lhsBh": np.ascontiguousarray(lhsB, dtype=bf16),
            "rhsAh": rhsAh,
            "rhsBh": rhsBh,
            "pnA": pnA_c,
            "pnB": pnB_c,
            "bthA": np.ascontiguousarray(TH - pnA_c, dtype=np.float32),
            "bthB": np.ascontiguousarray(TH - pnB_c, dtype=np.float32),
            "ones": ones_in,
        })
    return in_maps


def _exact_host_reference(F0, F1, matches, sel0, sel1):
    """Bit-faithful numpy port of the oracle, used only as a fallback when a
    nonzero hardest-negative sum is observed (mask handling then matters)."""
    hash_seed = max(F0.shape[0], F1.shape[0])
    pos_ind0 = matches[:, 0].astype(np.int64)
    pos_ind1 = matches[:, 1].astype(np.int64)
    posF0, posF1 = F0[pos_ind0], F1[pos_ind1]
    subF0, subF1 = F0[sel0], F1[sel1]

    def pd(A, B):
        d2 = ((A * A).sum(1)[:, None] + (B * B).sum(1)[None, :]
              - 2.0 * (A @ B.T))
        return np.sqrt(np.maximum(d2, 0.0) + 1e-7)

    D01 = pd(posF0, subF1)
    D10 = pd(posF1, subF0)
    D01min, D10min = D01.min(1), D10.min(1)
    D01ind = np.asarray(sel1)[np.argmin(D01, 1)].astype(np.int64)
    D10ind = np.asarray(sel0)[np.argmin(D10, 1)].astype(np.int64)
    pos_keys = pos_ind0 + pos_ind1 * hash_seed
    mask0 = ~np.isin(pos_ind0 + D01ind * hash_seed, pos_keys)
    mask1 = ~np.isin(D10ind + pos_ind1 * hash_seed, pos_keys)
    pos_loss = np.mean(np.maximum(((posF0 - posF1) ** 2).sum(1) - POS_THRESH, 0))
    n0 = np.maximum(NEG_THRESH - D01min, 0) ** 2
    n1 = np.maximum(NEG_THRESH - D10min, 0) ** 2
    neg0 = (n0 * mask0).sum() / max(mask0.sum(), 1)
    neg1 = (n1 * mask1).sum() / max(mask1.sum(), 1)
    return np.float32(pos_loss + (neg0 + neg1) / 2.0)


def kernel(F0, F1, matches, sel0, sel1):
    global _CACHED_NC, LAST_RESULTS
    F0 = np.ascontiguousarray(np.asarray(F0), dtype=np.float32)
    F1 = np.ascontiguousarray(np.asarray(F1), dtype=np.float32)
    matches = np.asarray(matches)
    sel0 = np.asarray(sel0)
    sel1 = np.asarray(sel1)
    assert F0.shape == (N_PTS, D) and matches.shape == (P, 2)
    assert sel0.shape == (M,) and sel1.shape == (M,)

    in_maps = _prep_inputs(F0, F1, matches, sel0, sel1)
    if _CACHED_NC is None:
        _CACHED_NC = _build_nc()
    try:
        res = run_bass_kernel_spmd(_CACHED_NC, in_maps, list(range(N_CORES)))
    except Exception:
        # a wedged NeuronCore (e.g. NRT_EXEC_UNIT_UNRECOVERABLE from an
        # earlier crashed session) is recoverable via the axon reset call
        try:
            import ctypes

            lib = ctypes.CDLL("/opt/axon/libaxon_pjrt.so")
            lib.axon_reset.restype = ctypes.c_int64
            lib.axon_reset()
        except Exception:
            pass
        res = run_bass_kernel_spmd(_CACHED_NC, in_maps, list(range(N_CORES)))
    LAST_RESULTS = res
    outs = np.stack([r["out"] for r in res.results])   # (8, 1, 2)
    pos_sum = float(outs[:, 0, 0].sum())
    flag = float(outs[:, 0, 1].sum())
    if flag != 0.0:
        # hardest negatives (or the relu certificate) crossed the threshold:
        # the pair-mask now matters; recompute exactly on host.
        return _exact_host_reference(F0, F1, matches, sel0, sel1)
    return np.float32(pos_sum / P)


# revision 7
# speedup vs baseline: 1.0665x; 1.0665x over previous
"""ContrastiveHardestNegativeLoss on 8 Trainium2 NeuronCores (Bass/Tile).

Strategy (per sharding hint): shard the positive-pair (row) dimension of the
P x M distance matrices across the 8 cores. Each core receives:
  - its slice of the gathered pos features, transposed + augmented:
      lhs[d, i] = posF[i, d] for d < 32,  lhs[32, i] = 1.0
  - the full gathered sub features, transposed, scaled and augmented:
      rhs[d, c] = -2 * subF[c, d] for d < 32,  rhs[32, c] = |subF[c]|^2
  so a single PE matmul produces q[i, c] = |subF[c]|^2 - 2 <posF[i], subF[c]>,
  i.e. the squared distance minus the (row-constant) |posF[i]|^2 term.

The hardest-negative terms are exactly zero whenever every distance exceeds
NEG_THRESH (true with huge margin here: min distance ~2.9 vs 1.4), in which
case the pair-mask cannot affect the result. The kernel therefore only needs
a *certificate* that no squared distance falls below a conservative threshold
TH = 3.0 > NEG_THRESH^2 + bf16 error margin; if the certificate fails it
falls back to an exact host recompute.

PSUM egress (the roofline here: only ScalarE and VectorE have PSUM read
ports) is organized in pairs: ScalarE copies the EVEN granule of each pair
to SBUF (a plain COPY - activations with accum_out cost an extra ~182ns
ACTIVATION_READ_ACCUMULATOR on ScalarE), then the 2-stream custom DVE min op
consumes (odd PSUM, even copy) at 2 elements/cycle with a free per-partition
running-min accumulator. 12 of the 256 granules instead go through a ScalarE
relu((TH-|p_i|^2)-q) certificate (SUM-accumulate must be exactly 0), sized
and spread (singly, between pairs) to fill ScalarE's idle time without
bubbling VectorE. Output per core: [pos_sum, flag]; flag must be exactly 0.
"""

import numpy as np

import concourse.bacc as bacc
import concourse.mybir as mybir
import concourse.tile as tile
from concourse.bass_utils import run_bass_kernel_spmd

N_CORES = 8
N_PTS = 100000
D = 32
P = 16384
M = 8192
P_LOC = P // N_CORES            # 2048 rows per core
RT = P_LOC // 128               # 16 row tiles
CHUNK = 1024                    # q columns per PSUM granule (2 banks)
NCH = M // CHUNK                # 8 chunks
KA = D + 1                      # contraction dim with augmentation row
POS_THRESH = 0.1
NEG_THRESH = 1.4
TH = 3.0                        # certificate threshold on d^2 (vs
                                # NEG_THRESH^2=1.96; margin covers bf16 error)

# block-halves (pr, mi, half) whose k=2 and k=5 granules go to the ScalarE
# relu path instead of a copy/min2 pair: 6 of 32 -> 12 relu granules, which
# balances S (122 copies + 12 relus + pos) against V (122 min2 pairs).
RELU_BH = {2, 8, 13, 18, 24, 29}

F32 = mybir.dt.float32
BF16 = mybir.dt.bfloat16
AX = mybir.AxisListType
ALU = mybir.AluOpType
ACT = mybir.ActivationFunctionType

_CACHED_NC = None
LAST_RESULTS = None            # test.py reads .exec_time_ns after a traced run


def _register_const(nc, value):
    t = nc.alloc_sbuf_tensor(f"const-float32-{value}", [128, 1], F32)
    nc.gpsimd.memset(t.ap(), value)
    nc.const_aps.aps[(F32, value)] = t.ap()


def _register_min2():
    """Custom DVE op: out = min(in0, in1) elementwise, accum_out[p] =
    min(s0, min_k out[p, k]). Consumes TWO streams per cycle (rd0 + rd1),
    doubling reduction throughput vs stock tensor_reduce (which is capped at
    one element/lane/cycle). Registered at runtime into dve_ops.OPS so the
    per-NEFF DVE table generator can resolve it by name."""
    import concourse.dve_ops as dops
    from concourse.dve_spec import C0, Spec, Src0, Src1, _has_src1, lower, minn
    from concourse.dve_uop import DveOpSpec

    name = "MIN2_STREAMS_ANT"
    for op in dops.OPS:
        if op.name == name:
            return op

    def ref(in0, in1, s0, s1, imm2):
        b = np.minimum(in0, in1).astype(np.float32)
        acc = np.minimum(b.reshape(b.shape[0], -1).min(-1, keepdims=True),
                         np.asarray(s0, np.float32).reshape(-1, 1))
        return b, acc

    spec = Spec(body=minn(Src0, Src1), accum=minn, accum_init=C0, reference=ref)
    row = dops._CUSTOM_DVE_ROW_BASE + len(dops.OPS)
    shas = {}
    for ver in ("v3", "v4"):
        uops = lower(spec, ver=ver)
        shas[ver] = DveOpSpec(name=name, opcode=row, uops=uops,
                              rd1_en=_has_src1(spec)).sha(ver)
    op = dops.DveOp(name, spec, subdim=False, uops_sha=shas)
    dops.OPS.append(op)
    dops.CUSTOM_DVE_SPECS[name] = spec
    dops._SUB_OPCODE_FOR_NAME[name] = row
    return op


def _build_nc():
    min2 = _register_min2()
    nc = bacc.Bacc("TRN2", debug=False, target_bir_lowering=False,
                   num_devices=N_CORES)
    # Const APs are memset once here and first read much later; no barrier
    # needed.
    for v in (-POS_THRESH, TH):
        _register_const(nc, v)
    # fp32 pos-pair operands (positive loss needs full precision);
    # bf16 copies feed the distance-matrix matmuls (fp32 PE matmul streams at
    # 1/4 rate; bf16 error on a distance is ~0.05 vs a 1.5 threshold margin).
    lhsA = nc.dram_tensor("lhsA", [KA, P_LOC], F32, kind="ExternalInput").ap()
    lhsB = nc.dram_tensor("lhsB", [KA, P_LOC], F32, kind="ExternalInput").ap()
    lhsAh = nc.dram_tensor("lhsAh", [KA, P_LOC], BF16, kind="ExternalInput").ap()
    lhsBh = nc.dram_tensor("lhsBh", [KA, P_LOC], BF16, kind="ExternalInput").ap()
    rhsAh = nc.dram_tensor("rhsAh", [KA, M], BF16, kind="ExternalInput").ap()
    rhsBh = nc.dram_tensor("rhsBh", [KA, M], BF16, kind="ExternalInput").ap()
    pnA = nc.dram_tensor("pnA", [128, RT], F32, kind="ExternalInput").ap()
    pnB = nc.dram_tensor("pnB", [128, RT], F32, kind="ExternalInput").ap()
    bthA = nc.dram_tensor("bthA", [128, RT], F32, kind="ExternalInput").ap()
    bthB = nc.dram_tensor("bthB", [128, RT], F32, kind="ExternalInput").ap()
    ones = nc.dram_tensor("ones", [128, 1], F32, kind="ExternalInput").ap()
    outd = nc.dram_tensor("out", [1, 2], F32, kind="ExternalOutput").ap()

    with tile.TileContext(nc) as tc:
        with (
            tc.tile_pool(name="ops", bufs=1) as ops,
            tc.tile_pool(name="wk", bufs=2) as wk,
            tc.tile_pool(name="ps", bufs=4, space="PSUM") as ps,
        ):
            t_lhsA = ops.tile([KA, P_LOC], F32, tag="lhsA")
            t_lhsB = ops.tile([KA, P_LOC], F32, tag="lhsB")
            # bf16 operands are loaded TWICE: rows 0..32 and rows 64..96, so
            # two row-tiles' matmuls can run concurrently on the two 64-row
            # groups of the PE array (K=33 rounds up to a 64-row group).
            t_lhsAh = ops.tile([128, P_LOC], BF16, tag="lhsAh")
            t_lhsBh = ops.tile([128, P_LOC], BF16, tag="lhsBh")
            t_rhsAh = ops.tile([128, M], BF16, tag="rhsAh")
            t_rhsBh = ops.tile([128, M], BF16, tag="rhsBh")
            t_pnAB = ops.tile([128, 2 * RT], F32, tag="pnAB")
            t_bthA = ops.tile([128, RT], F32, tag="bthA")
            t_bthB = ops.tile([128, RT], F32, tag="bthB")
            t_ones = ops.tile([128, 1], F32, tag="ones")
            t_racc = ops.tile([128, 16], F32, tag="racc")
            # per-pair running-min columns: col = mi*64 + r*4 + pair_slot
            t_cmin = ops.tile([128, 128], F32, tag="cmin")

            # operand loads strictly in consumption order. The four tiles
            # the first matmuls need go out in parallel on four different
            # DMA rings so their descriptors don't serialize; the bulk of
            # each tensor follows in one coarse descriptor on the sync ring.
            sl0 = slice(0, CHUNK)
            slr = slice(CHUNK, M)
            nc.sync.dma_start(t_rhsAh[0:KA, sl0], rhsAh[:, sl0])
            nc.scalar.dma_start(t_rhsAh[64:64 + KA, sl0], rhsAh[:, sl0])
            nc.gpsimd.dma_start(t_lhsAh[0:KA, :], lhsAh[:])
            nc.scalar.dma_start(t_lhsAh[64:64 + KA, :], lhsAh[:])
            for base in (0, 64):
                nc.sync.dma_start(t_rhsAh[base:base + KA, slr], rhsAh[:, slr])
            for base in (0, 64):
                nc.sync.dma_start(t_lhsBh[base:base + KA, :], lhsBh[:])
            for base in (0, 64):
                nc.sync.dma_start(t_rhsBh[base:base + KA, :], rhsBh[:])
            # aux + fp32 operands go over the (otherwise idle) GpSimd DMA
            # ring so they don't queue behind the bf16 stream on the sync
            # ring; the fp32 lhs feed the pos-path prep that runs early.
            nc.gpsimd.dma_start(t_lhsA[:], lhsA[:])
            nc.gpsimd.dma_start(t_lhsB[:], lhsB[:])
            nc.gpsimd.dma_start(t_bthA[:], bthA[:])
            nc.gpsimd.dma_start(t_bthB[:], bthB[:])
            nc.gpsimd.dma_start(t_pnAB[:, 0:RT], pnA[:])
            nc.gpsimd.dma_start(t_pnAB[:, RT:2 * RT], pnB[:])
            nc.gpsimd.dma_start(t_ones[:], ones[:])

            # pos-path prep (VectorE sub + ScalarE square) early: fills the
            # consumers' startup idle while the first bf16 chunks stream in.
            t_onesh = ops.tile([KA, 1], BF16, tag="onesh")
            nc.gpsimd.memset(t_onesh[:], 1.0)
            t_dif = ops.tile([KA, P_LOC], F32, tag="dif")
            nc.vector.tensor_tensor(t_dif[:], t_lhsA[:], t_lhsB[:], ALU.subtract)
            t_difsq = ops.tile([KA, P_LOC], BF16, tag="difsq")
            nc.scalar.activation(t_difsq[:], t_dif[:], ACT.Square)

            # racc columns beyond rcol must stay zero; cmin pair-slots that
            # relu block-halves skip must not poison the epilogue min.
            nc.gpsimd.memset(t_racc[:], 0.0)
            nc.gpsimd.memset(t_cmin[:], 3.0e38)

            t_outsb = wk.tile([1, 2], F32, tag="outsb")
            t_posacc = ops.tile([1, 2], F32, tag="posacc")

            # ---- the two distance matrices ----
            # Row-tiles processed in pairs (PE row-groups 0 and 64). Granule
            # pairing per block-half: normal (k0,k1)(k2,k3)(k4,k5)(k6,k7);
            # relu block-halves (k0,k1)(k3,k4)(k6,k7) with k2, k5 on the
            # ScalarE relu path (spread singly so the copies V depends on
            # aren't queued behind relus).
            rcol = 0
            for pr in range(RT // 2):
                for mi, (t_lhs, t_rhs, t_bth) in enumerate((
                    (t_lhsAh, t_rhsAh, t_bthA),
                    (t_lhsBh, t_rhsBh, t_bthB),
                )):
                    held_qc = {}
                    pslot = {0: 0, 1: 0}
                    pstart = {}
                    for k in range(NCH):
                        qs = {}
                        for half in (0, 1):
                            qs[half] = ps.tile([128, CHUNK], F32, tag="q",
                                               name=f"q{half}")
                        # emit the two halves' matmuls interleaved by 512-col
                        # piece so consecutive PE ops alternate row-groups
                        # (lets each LDWEIGHTS pull ahead during the other
                        # group's matmul).
                        for j in range(CHUNK // 512):
                            for half in (0, 1):
                                r = 2 * pr + half
                                base = 64 * half
                                w = t_lhs[base:base + KA,
                                          r * 128:(r + 1) * 128]
                                c0 = k * CHUNK + j * 512
                                nc.tensor.matmul(
                                    qs[half][:, j * 512:(j + 1) * 512], w,
                                    t_rhs[base:base + KA, c0:c0 + 512])
                        for half in (0, 1):
                            r = 2 * pr + half
                            bh = (pr * 2 + mi) * 2 + half
                            rblk = bh in RELU_BH
                            q = qs[half]
                            if rblk and k in (2, 5):
                                junk_r = wk.tile([128, CHUNK], BF16,
                                                 tag="junkr", bufs=2)
                                nc.scalar.activation(
                                    junk_r[:], q[:], ACT.Relu,
                                    bias=t_bth[:, r:r + 1], scale=-1.0,
                                    accum_out=t_racc[:, rcol:rcol + 1])
                                rcol += 1
                            elif half not in pstart:
                                # first granule of a pair: ScalarE copy to
                                # SBUF now; the copy overlaps the next
                                # granule's matmuls.
                                qc = wk.tile([128, CHUNK], F32, tag="qc",
                                             bufs=6)
                                nc.scalar.copy(qc[:], q[:])
                                held_qc[half] = qc
                                pstart[half] = k
                            else:
                                junk = wk.tile([128, CHUNK], F32, tag="junk",
                                               bufs=2)
                                col = (mi * 64 + r * 4 + pslot[half])
                                nc.vector._custom_dve(
                                    min2, out=junk[:], in0=q[:],
                                    in1=held_qc[half][:], s0=3.0e38,
                                    accum_out=t_cmin[:, col:col + 1])
                                pslot[half] += 1
                                del pstart[half]
                    # positive-pair loss, emitted early in the stream so its
                    # ScalarE relus land inside the main span (not the tail):
                    # relu(sum_d (p0-p1)^2 - 0.1) summed via accum_out. The
                    # per-pair squares are summed on the PE via a ones-matmul
                    # in bf16 (rounding perturbs the loss ~1e-5 relative).
                    if pr == 1 and mi == 1:
                        for h in range(2):
                            pp = ps.tile([128, CHUNK], F32, tag="q",
                                         name="pp")
                            for j in range(2):
                                c0 = (2 * h + j) * 512
                                nc.tensor.matmul(
                                    pp[0:1, j * 512:(j + 1) * 512],
                                    t_onesh[:], t_difsq[:, c0:c0 + 512])
                            junkp = wk.tile([1, CHUNK], BF16, tag="junkp")
                            nc.scalar.activation(
                                junkp[:], pp[0:1, :], ACT.Relu,
                                bias=-POS_THRESH,
                                accum_out=t_posacc[0:1, h:h + 1])

            # ---- fused epilogue over both matrices: per-row min -> d2 ->
            # relu(TH - d2), combined with the S-path relu certificate into
            # one nonnegative flag that must be exactly 0.
            nc.vector.tensor_reduce(
                out=t_outsb[0:1, 0:1], in_=t_posacc[:], axis=AX.X,
                op=ALU.add)
            minq = wk.tile([128, 2 * RT], F32, tag="minq")
            nc.vector.tensor_reduce(
                out=minq[:],
                in_=t_cmin.rearrange("p (m k) -> p m k", k=4),
                axis=AX.X, op=ALU.min)
            d2 = wk.tile([128, 2 * RT], F32, tag="d2")
            nc.vector.tensor_tensor(d2[:], minq[:], t_pnAB[:], ALU.add)
            junkq = wk.tile([128, 2 * RT], BF16, tag="junkq")
            t_flag = wk.tile([128, 1], F32, tag="flag")
            nc.scalar.activation(junkq[:], d2[:], ACT.Relu, bias=TH,
                                 scale=-1.0, accum_out=t_flag[:])
            rs = wk.tile([128, 1], F32, tag="rs")
            nc.vector.tensor_reduce(out=rs[:], in_=t_racc[:], axis=AX.X,
                                    op=ALU.add)
            comb = wk.tile([128, 1], F32, tag="comb")
            nc.vector.tensor_tensor(comb[:], t_flag[:], rs[:], ALU.add)
            fp = ps.tile([1, 1], F32, tag="q")
            nc.tensor.matmul(fp[:], comb[:], t_ones[:])
            nc.scalar.copy(t_outsb[0:1, 1:2], fp[0:1, 0:1])

            nc.sync.dma_start(outd[:], t_outsb[:])

    nc.compile()
    return nc


def _prep_inputs(F0, F1, matches, sel0, sel1):
    posF0 = F0[matches[:, 0]]
    posF1 = F1[matches[:, 1]]
    subF0 = F0[sel0]
    subF1 = F1[sel1]
    import ml_dtypes

    bf16 = ml_dtypes.bfloat16
    ones_col = np.ones((1, P_LOC), np.float32)
    rhsA = np.ascontiguousarray(
        np.concatenate([-2.0 * subF1.T, (subF1 * subF1).sum(1)[None, :]], 0),
        dtype=np.float32)
    rhsB = np.ascontiguousarray(
        np.concatenate([-2.0 * subF0.T, (subF0 * subF0).sum(1)[None, :]], 0),
        dtype=np.float32)
    rhsAh = np.ascontiguousarray(rhsA, dtype=bf16)
    rhsBh = np.ascontiguousarray(rhsB, dtype=bf16)
    ones_in = np.ones((128, 1), np.float32)
    in_maps = []
    for c in range(N_CORES):
        sl = slice(c * P_LOC, (c + 1) * P_LOC)
        p0, p1 = posF0[sl], posF1[sl]
        lhsA = np.ascontiguousarray(
            np.concatenate([p0.T, ones_col], 0), dtype=np.float32)
        lhsB = np.ascontiguousarray(
            np.concatenate([p1.T, ones_col], 0), dtype=np.float32)
        pnA_c = np.ascontiguousarray(
            (p0 * p0).sum(1).reshape(RT, 128).T, dtype=np.float32)
        pnB_c = np.ascontiguousarray(
            (p1 * p1).sum(1).reshape(RT, 128).T, dtype=np.float32)
        in_maps.append({
            "lhsA": lhsA,
            "lhsB": lhsB,
            "lhsAh": np.ascontiguousarray(lhsA, dtype=bf16),
            "lhsBh": np.ascontiguousarray(lhsB, dtype=bf16),
            "rhsAh": rhsAh,
            "rhsBh": rhsBh,
            "pnA": pnA_c,
            "pnB": pnB_c,
            "bthA": np.ascontiguousarray(TH - pnA_c, dtype=np.float32),
            "bthB": np.ascontiguousarray(TH - pnB_c, dtype=np.float32),
            "ones": ones_in,
        })
    return in_maps


def _exact_host_reference(F0, F1, matches, sel0, sel1):
    """Bit-faithful numpy port of the oracle, used only as a fallback when a
    nonzero hardest-negative sum is observed (mask handling then matters)."""
    hash_seed = max(F0.shape[0], F1.shape[0])
    pos_ind0 = matches[:, 0].astype(np.int64)
    pos_ind1 = matches[:, 1].astype(np.int64)
    posF0, posF1 = F0[pos_ind0], F1[pos_ind1]
    subF0, subF1 = F0[sel0], F1[sel1]

    def pd(A, B):
        d2 = ((A * A).sum(1)[:, None] + (B * B).sum(1)[None, :]
              - 2.0 * (A @ B.T))
        return np.sqrt(np.maximum(d2, 0.0) + 1e-7)

    D01 = pd(posF0, subF1)
    D10 = pd(posF1, subF0)
    D01min, D10min = D01.min(1), D10.min(1)
    D01ind = np.asarray(sel1)[np.argmin(D01, 1)].astype(np.int64)
    D10ind = np.asarray(sel0)[np.argmin(D10, 1)].astype(np.int64)
    pos_keys = pos_ind0 + pos_ind1 * hash_seed
    mask0 = ~np.isin(pos_ind0 + D01ind * hash_seed, pos_keys)
    mask1 = ~np.isin(D10ind + pos_ind1 * hash_seed, pos_keys)
    pos_loss = np.mean(np.maximum(((posF0 - posF1) ** 2).sum(1) - POS_THRESH, 0))
    n0 = np.maximum(NEG_THRESH - D01min, 0) ** 2
    n1 = np.maximum(NEG_THRESH - D10min, 0) ** 2
    neg0 = (n0 * mask0).sum() / max(mask0.sum(), 1)
    neg1 = (n1 * mask1).sum() / max(mask1.sum(), 1)
    return np.float32(pos_loss + (neg0 + neg1) / 2.0)


def kernel(F0, F1, matches, sel0, sel1):
    global _CACHED_NC, LAST_RESULTS
    F0 = np.ascontiguousarray(np.asarray(F0), dtype=np.float32)
    F1 = np.ascontiguousarray(np.asarray(F1), dtype=np.float32)
    matches = np.asarray(matches)
    sel0 = np.asarray(sel0)
    sel1 = np.asarray(sel1)
    assert F0.shape == (N_PTS, D) and matches.shape == (P, 2)
    assert sel0.shape == (M,) and sel1.shape == (M,)

    in_maps = _prep_inputs(F0, F1, matches, sel0, sel1)
    if _CACHED_NC is None:
        _CACHED_NC = _build_nc()
    try:
        res = run_bass_kernel_spmd(_CACHED_NC, in_maps, list(range(N_CORES)))
    except Exception:
        # a wedged NeuronCore (e.g. NRT_EXEC_UNIT_UNRECOVERABLE from an
        # earlier crashed session) is recoverable via the axon reset call
        try:
            import ctypes

            lib = ctypes.CDLL("/opt/axon/libaxon_pjrt.so")
            lib.axon_reset.restype = ctypes.c_int64
            lib.axon_reset()
        except Exception:
            pass
        res = run_bass_kernel_spmd(_CACHED_NC, in_maps, list(range(N_CORES)))
    LAST_RESULTS = res
    outs = np.stack([r["out"] for r in res.results])   # (8, 1, 2)
    pos_sum = float(outs[:, 0, 0].sum())
    flag = float(outs[:, 0, 1].sum())
    if flag != 0.0:
        # hardest negatives (or the relu certificate) crossed the threshold:
        # the pair-mask now matters; recompute exactly on host.
        return _exact_host_reference(F0, F1, matches, sel0, sel1)
    return np.float32(pos_sum / P)
